# revision 81
# baseline (speedup 1.0000x reference)
"""Causal GQA attention on 8 Trainium2 NeuronCores.

Problem: q [2048, 32, 128], k/v [2048, 8, 128] fp32; out = causal softmax
attention with GQA (4 query heads per KV head), scale 1/sqrt(128).

Sharding: tensor-parallel on the head axis — core c gets query heads
4c..4c+3 and KV head c (GQA groups stay co-located). No collectives.

Per-core kernel (host preps layout: Q^T/K^T transposed + bf16 cast, part of
the sharding step):
  - scores computed TRANSPOSED, ST[k, (head, q)] = K^T-tile contracted with
    Q^T over d: one 128x512 bf16 matmul per (qblock, ktile) — the 4 query
    heads of the GQA group share the K tile, packed into the moving operand.
  - exp split across TWO engines (the scalar/ACT engine is otherwise the
    bottleneck): ACT groups use the exact table exp (scale folded in);
    DVE groups use a Schraudolph bit-trick — i16 = s*alpha + beta computed
    by one tensor_scalar (fp32 in, int16 out), whose bits ARE the bf16
    exp(s*SCALE) approximation (rel err ~1.8% rms, cancels in softmax).
    Groups are assigned by greedy load balancing; diagonal groups must use
    ACT (the -1e9-masked scores only behave under a real exp).
  - causal diagonal masked by zeroing exp entries via gpsimd affine_select.
  - PV: out[q, 0:129] += expST[:, hchunk]^T @ [V | 1] accumulated in PSUM
    over k-tiles; column 128 accumulates the softmax denominator for free.
  - normalize on the vector engine (reciprocal + broadcast multiply).
Query blocks run in descending causal-length order so the PE sees a dense
matmul stream immediately (HAM warms early).
"""
from contextlib import ExitStack

import numpy as np
import ml_dtypes

import concourse.tile as tile
import concourse.mybir as mybir
from concourse import bacc
from concourse.bass_utils import run_bass_kernel_spmd

F32 = mybir.dt.float32
BF16 = mybir.dt.bfloat16
I16 = mybir.dt.int16

S = 2048
H = 32          # total query heads
KVH = 8         # total KV heads
HQ = 4          # query heads per core
D = 128
NT = S // 128   # 16 query/key tiles
NCORES = 8
SCALE = 0.08838834764831845
GRP = 2         # k-tiles per exp batch

# Schraudolph bf16 exp-from-bits: i16 = s*ALPHA16 + BETA16, bits are bf16
# exp(s*SCALE). c=-7.0 tuned for truncating fp32->int16 conversion.
ALPHA16 = SCALE * 1.4426950408889634 * 128.0
BETA16 = 127.0 * 128.0 - 7.0

# engine-balance costs (ns, HW-measured) for greedy exp-group assignment
ACT_NS_PER_KT = 470.0
DVE_NS_PER_KT = 530.0
ACT_OVH = 160.0
DVE_OVH = 140.0


def _groups(qb):
    nkt = qb + 1
    return [(qb, g) for g in range((nkt + GRP - 1) // GRP)]


def _group_kts(qb, g):
    nkt = qb + 1
    return [i for i in range(GRP * g, GRP * g + GRP) if i < nkt]


def _slot_order():
    """Group issue order: big qblocks first (descending). The last few
    qblocks use 1-ktile groups so the final exps spread across engines."""
    slots = []
    for qb in range(NT - 1, -1, -1):
        if qb >= 6:
            slots.extend(_groups(qb))
        else:
            slots.extend((qb, -(kt + 1)) for kt in range(qb + 1))
    return slots


def _plan_groups(slots):
    """Assign each (qb, group) exp batch to 'act' or 'dve', greedily
    balancing projected engine time. Diagonal groups stay on ACT."""
    plan = {}
    act_t, dve_t = 0.0, 0.0
    for qb, g in slots:
        nkt = qb + 1
        if g < 0:
            n = 1
            has_diag = (-g - 1) == qb
        else:
            n = min(GRP, nkt - GRP * g)
            has_diag = (GRP * g + n - 1) == qb
        ca = n * ACT_NS_PER_KT + ACT_OVH
        cd = n * DVE_NS_PER_KT + DVE_OVH
        if act_t + ca <= dve_t + cd:
            plan[(qb, g)] = 'act'
            act_t += ca
        else:
            plan[(qb, g)] = 'dve'
            dve_t += cd
        if has_diag:
            act_t += 550.0   # pv drain copy (accumulator 0) rides ACT
            dve_t += 430.0   # pv drain copy (accumulator 1) rides DVE
    return plan


def _build_nc():
    nc = bacc.Bacc("TRN2", target_bir_lowering=False)
    qtd = nc.dram_tensor("qt", [128, HQ, NT, 128], BF16, kind="ExternalInput")
    ktd = nc.dram_tensor("kt", [128, NT, 128], BF16, kind="ExternalInput")
    vd = nc.dram_tensor("v", [S, D], BF16, kind="ExternalInput")
    out = nc.dram_tensor("out", [S, HQ, D], F32, kind="ExternalOutput")

    v3 = vd.ap().rearrange("(t p) d -> p t d", p=128)
    slots = _slot_order()
    plan = _plan_groups(slots)

    with tile.TileContext(nc) as tc, ExitStack() as ctx:
        big = ctx.enter_context(tc.tile_pool(name="big", bufs=1))
        qt = big.tile([128, HQ, NT, 128], BF16)  # [d, h, g(=NT-1-qb), q]
        kt = big.tile([128, NT, 128], BF16)      # [d, kblk, k]
        v1 = big.tile([128, NT, 132], BF16)      # [k, kblk, 129(+pad)]

        # warm the ACT exp table early
        dummy = big.tile([128, 1], F32)
        nc.vector.memset(dummy[:], 0.0)
        dume = big.tile([128, 1], F32)
        nc.scalar.activation(dume[:], dummy[:],
                             mybir.ActivationFunctionType.Exp)

        # HAM warm-up: dense dummy matmuls while DMAs stream in. dw is
        # deliberately uninitialized (results land in a dead PSUM tile) so
        # the warm-up isn't gated on a memset clearing the entry barrier.
        dw = big.tile([128, 512], BF16)
        nc.gpsimd.memset(dw[:], 0.0)
        with tc.tile_pool(name="dpool", bufs=1, space="PSUM") as dpool:
            dps = dpool.tile([128, 512], F32)
            for _ in range(10):
                nc.tensor.matmul(dps[:, 0:256], dw[:, :128], dw[:, 0:256],
                                 start=True, stop=True)

        # loads: sync ring gets K + V, scalar ring Q (earliest blocks first)
        nc.sync.dma_start(kt[:, 0:2, :], ktd[:, 0:2, :])
        nc.scalar.dma_start(qt[:, :, 0:1, :], qtd[:, :, 0:1, :])
        nc.sync.dma_start(kt[:, 2:8, :], ktd[:, 2:8, :])
        nc.scalar.dma_start(qt[:, :, 1:2, :], qtd[:, :, 1:2, :])
        nc.sync.dma_start(v1[:, 0:8, 0:128], v3[:, 0:8, :])
        nc.scalar.dma_start(qt[:, :, 2:4, :], qtd[:, :, 2:4, :])
        nc.sync.dma_start(kt[:, 8:16, :], ktd[:, 8:16, :])
        nc.scalar.dma_start(qt[:, :, 4:8, :], qtd[:, :, 4:8, :])
        nc.sync.dma_start(v1[:, 8:16, 0:128], v3[:, 8:16, :])
        nc.scalar.dma_start(qt[:, :, 8:12, :], qtd[:, :, 8:12, :])
        nc.scalar.dma_start(qt[:, :, 12:16, :], qtd[:, :, 12:16, :])
        nc.vector.memset(v1[:, :, 128:129], 1.0)

        def do_normalize(qb, pvs):
            # out[h] = pvs[h, 0:128] / pvs[h, 128] on gpsimd (Q7), from the
            # SBUF copy of the PSUM accumulators
            ot = outp.tile([128, HQ, 128], F32, tag="ot", name="ot")
            for i in range(2):
                for j in range(2):
                    nc.gpsimd.normalize_recip(
                        ot[:, 2 * i + j, :], pvs[:, i, j, 0:128],
                        pvs[:, i, j, 128:129])
            nc.sync.dma_start(out[qb * 128:(qb + 1) * 128, :, :], ot[:])

        with tc.tile_pool(name="stp", bufs=3, space="PSUM") as stp, \
             tc.tile_pool(name="pvp", bufs=1, space="PSUM") as pvp, \
             tc.tile_pool(name="expp", bufs=8) as expp, \
             tc.tile_pool(name="pvsb", bufs=3) as pvsb, \
             tc.tile_pool(name="outp", bufs=4) as outp:

            def emit_pv(w):
                # PV matmuls for a group whose exp was issued LAG slots ago
                for j, kt_i in enumerate(w['kts']):
                    for h in range(HQ):
                        nc.tensor.matmul(
                            w['pv'][h // 2][:, h % 2], w['e2'][:, j, h, :],
                            v1[:, kt_i, 0:129],
                            start=(kt_i == 0 and h % 2 == 0),
                            stop=(kt_i == w['qb'] and h % 2 == 1))

            def emit_copy(pv_pair, pvs):
                # drain final accumulators PSUM->SBUF for gpsimd normalize
                nc.scalar.copy(pvs[:, 0, :, 0:129], pv_pair[0][:])
                nc.vector.tensor_scalar_add(
                    pvs[:, 1, :, 0:129], pv_pair[1][:], 0.0)

            from collections import deque
            LAG = 7            # PV trails QK/exp by this many group slots
            pvq = deque()      # groups awaiting PV emission
            normq = deque()    # (qb, pvs) awaiting gpsimd normalize
            pvmap = {}         # qb -> pv accumulator pair

            def pop_pv():
                w = pvq.popleft()
                emit_pv(w)
                if w['kts'][-1] == w['qb']:
                    if w['qb'] >= 4:
                        # final group of its qb: drain the accumulators now
                        # (ordered right after the closing PV matmuls)
                        emit_copy(w['pv'], w['pvs'])
                        normq.append((w['qb'], w['pvs']))
                        # normalize from a qb back: deps are settled, so
                        # the gpsimd queue never head-blocks on it
                        if len(normq) > 1:
                            do_normalize(*normq.popleft())
                    else:
                        # tail qbs: normalize directly from PSUM on the DVE
                        # (idle by now) so gpsimd can drain early
                        while normq:
                            do_normalize(*normq.popleft())
                        qb, pv = w['qb'], w['pv']
                        ot = outp.tile([128, HQ, 128], F32, tag="ot",
                                       name="ott")
                        rl = outp.tile([128, HQ, 1], F32, tag="rl",
                                       name="rl")
                        for i in range(2):
                            nc.vector.reciprocal(rl[:, 2 * i:2 * i + 2, 0],
                                                 pv[i][:, :, 128])
                            nc.vector.tensor_tensor(
                                ot[:, 2 * i:2 * i + 2, :],
                                pv[i][:, :, 0:128],
                                rl[:, 2 * i:2 * i + 2, :].to_broadcast(
                                    (128, 2, 128)),
                                mybir.AluOpType.mult)
                        nc.sync.dma_start(
                            out[qb * 128:(qb + 1) * 128, :, :], ot[:])

            for si, (qb, g) in enumerate(slots):
                g_q = NT - 1 - qb  # index on the reversed qblk axis
                nkt = qb + 1
                kts = ([-g - 1] if g < 0 else
                       [i for i in range(GRP * g, GRP * g + GRP) if i < nkt])
                if kts[0] == 0:
                    pvmap[qb] = [pvp.tile([128, 2, 129], F32, tag=f"pvp{i}",
                                          name=f"pv{qb}_{i}")
                                 for i in range(2)]
                pv = pvmap[qb]
                n = len(kts)
                st2 = stp.tile([128, GRP, HQ, 128], F32, tag="st2")
                e2 = expp.tile([128, GRP, HQ, 128], BF16, tag="e2")
                for j, kt_i in enumerate(kts):
                    nc.tensor.matmul(
                        st2[:, j], kt[:, kt_i, :], qt[:, :, g_q, :],
                        start=True, stop=True)
                if si < 6:
                    # keep PE duty high through the pipeline-fill window so
                    # HAM doesn't re-throttle mid-body; garbage lands in the
                    # qb15 accumulators, wiped by their real start=True later
                    pvd = pvmap[NT - 1]
                    nc.tensor.matmul(
                        pvd[si % 2][:, (si // 2) % 2], dw[:, 0:128],
                        dw[:, 0:129], start=True, stop=True)
                if plan[(qb, g)] == 'act':
                    nc.scalar.activation(
                        e2[:, 0:n], st2[:, 0:n],
                        mybir.ActivationFunctionType.Exp, scale=SCALE)
                else:
                    nc.vector.tensor_scalar(
                        e2[:, 0:n].bitcast(I16), st2[:, 0:n],
                        ALPHA16, BETA16,
                        mybir.AluOpType.mult, mybir.AluOpType.add)
                if kts[-1] == qb:
                    # diagonal: zero exp where k_local > q_local
                    nc.gpsimd.affine_select(
                        out=e2[:, n - 1], in_=e2[:, n - 1],
                        compare_op=mybir.AluOpType.is_ge,
                        fill=0.0, base=0,
                        pattern=[[0, HQ], [1, 128]],
                        channel_multiplier=-1)
                w = {'qb': qb, 'kts': kts, 'e2': e2, 'pv': pv}
                if kts[-1] == qb:
                    w['pvs'] = pvsb.tile([128, 2, 2, 132], F32,
                                         tag="pvs", name="pvs")
                pvq.append(w)
                if len(pvq) > LAG:
                    pop_pv()
            while pvq:
                pop_pv()
            while normq:
                do_normalize(*normq.popleft())

    nc.finalize()
    return nc


_NC_CACHE = None


def kernel(q, k, v):
    global _NC_CACHE
    q = np.asarray(q, dtype=np.float32)
    k = np.asarray(k, dtype=np.float32)
    v = np.asarray(v, dtype=np.float32)
    assert q.shape == (S, H, D) and k.shape == (S, KVH, D)

    if _NC_CACHE is None:
        _NC_CACHE = _build_nc()
    nc = _NC_CACHE

    in_maps = []
    for c in range(NCORES):
        qs = q[:, c * HQ:(c + 1) * HQ, :].astype(ml_dtypes.bfloat16)
        # Q^T: [d, h, qblk, q] with the qblk axis reversed
        qtn = qs.transpose(2, 1, 0).reshape(D, HQ, NT, 128)[:, :, ::-1, :]
        ktn = k[:, c, :].astype(ml_dtypes.bfloat16).T.reshape(D, NT, 128)
        in_maps.append({
            "qt": np.ascontiguousarray(qtn),
            "kt": np.ascontiguousarray(ktn),
            "v": np.ascontiguousarray(v[:, c, :].astype(ml_dtypes.bfloat16)),
        })

    res = run_bass_kernel_spmd(nc, in_maps, core_ids=list(range(NCORES)))
    return np.concatenate([res.results[c]["out"] for c in range(NCORES)],
                          axis=1)
